# revision 7
# baseline (speedup 1.0000x reference)
"""Trainium2 Bass kernel for nn_MemoryUpdate (gated LIF memory update), v2.

Reference computation (fp32):
    k         = einsum('tbnd,od->tbno', kv, Wg)          # kv @ Wg^T
    gate_mean = mean_t'( k[t', b, nkv, d] )              # [Nkv, B, 1, D], Nkv == T
    update    = gate_mean[t, b, d] * q[t, b, n, d]       # broadcast over n
    spikes    = LIF over t: v' = (v + u)/2 ; s = v' >= 0.5 ; v = v' * (1 - s)

Shapes: q [4, 32, 1024, 512], kv [4, 32, 4, 512], Wg [512, 512] -> out [4, 32, 1024, 512].

v2 strategy (v1 was ~238 us, near the fp32 HBM roofline of ~187 us/core):
cut HBM traffic 2.7x and unlock 16-bit DVE perf modes.
  - d on partitions: host transposes q to [T, B, D, NQ]. The gate becomes a
    per-partition scalar, so the gate multiply fuses into the recurrence as
    scalar_tensor_tensor((q * g[P,1]) + w) — no broadcast tiles, no gate DMA.
  - q quantized host-side to int16 (scale 6000, max|q*S1| < 32767): halves
    the dominant read. Spikes written as uint8 (exact 0/1): quarter write.
    Per-core traffic 33.5+33.5 MB -> 16.8+8.4 MB.
  - int16 state with a doubling scale schedule S2[t] = 8192 << t: w_i stored
    at scale S2 *is* v/2 at scale 2*S2 = S2[t+1], so the LIF halving is free,
    each step has exactly one int16 rounding site (round-half-even, saturating
    — HW-verified), and precision doubles every step. Saturation at +/-32767
    only clips |v'| > 4 early (none exist) and |v'| > 2/1 late, where positive
    clips can't change the spike bit and negative clips are rare tails.
  - t=3 uses virtual scale 65536 via threshold 32767 (only s is needed).
  - spike threshold: ACT sigmoid(1e30 * a_i + (0.5 - TH)*1e30) -> uint8.
    Integer gap of 1 around TH makes the fma sign exact; sigmoid saturates
    to exact 0.0/1.0 (HW-verified probe).
Emulated end-to-end flip count vs fp32 reference: 463 of 67.1M outputs
(rel err 1.46e-2 < 2e-2 gate), matching HW cast semantics bit-for-bit.

Engine budget per core: DVE ~60 us (merged stt 2x_1P int16, w-op stt 2x,
t0 tensor_scalar 4x), ACT ~59 us (16 sigmoids [128,4096]), DMA ~70 us
(25.2 MB at ~355 GB/s). Target wall ~80-90 us.
"""

import sys

for p in ("/opt/trn_rl_repo", "/root/.axon_site/_ro/trn_rl_repo"):
    if p not in sys.path:
        sys.path.insert(0, p)

import numpy as np

import concourse.bass as bass
import concourse.mybir as mybir
import concourse.tile as tile
from concourse import bacc
from concourse.bass_utils import run_bass_kernel_spmd

# Problem constants (hardcoded per harness contract)
T, B, NQ, NKV, D = 4, 32, 1024, 4, 512
N_CORES = 8
B_LOC = B // N_CORES  # 4
P = 128               # partitions
C = D // P            # 4 d-chunks
NI = T * B_LOC * NKV  # 64 kvT columns
NG = B_LOC * NKV      # 16 gate rows per core

S1 = 6000.0                                    # q fixed-point scale
S2 = [8192.0, 16384.0, 32768.0, 65536.0]       # state scale per step (doubling)
TH = [4096.0, 8192.0, 16384.0, 32767.0]        # spike threshold in a_i units
BIG = 1.0e30

FP32 = mybir.dt.float32
I16 = mybir.dt.int16
U8 = mybir.dt.uint8
Alu = mybir.AluOpType
UNROLL = 4  # static inner unroll inside the timing-mode For_i loop


def build_kernel(repeats=1, timing_mode=False, ablate=frozenset()):
    nc = bacc.Bacc("TRN2", target_bir_lowering=False, debug=False,
                   num_devices=N_CORES)

    if timing_mode:
        # timing-only variant: big tensors live in internal DRAM so the wall
        # clock isn't dominated by host<->device transfers; the main body runs
        # `repeats` times in an on-device loop.
        qT = nc.dram_tensor("q_int", [T, B_LOC, D, NQ], I16).ap()
        out = nc.dram_tensor("out_int", [T, B_LOC, D, NQ], U8).ap()
        dummy = nc.dram_tensor("tiny_out", [P, 16], FP32, kind="ExternalOutput").ap()
    else:
        qT = nc.dram_tensor("qT", [T, B_LOC, D, NQ], I16, kind="ExternalInput").ap()
        out = nc.dram_tensor("out", [T, B_LOC, D, NQ], U8, kind="ExternalOutput").ap()
        dummy = None
    kvT = nc.dram_tensor("kvT", [D, NI], FP32, kind="ExternalInput").ap()
    # wgT is Wg^T host-scaled by 1/(2*T*S1): folds the t'-mean, the LIF /tau
    # and the int16 q descale into the gate.
    wgT = nc.dram_tensor("wgT", [D, D], FP32, kind="ExternalInput").ap()

    # d = c*128 + p on partitions; free = (c, n) -> per-partition runs of
    # 2 KiB (int16 q) / 1 KiB (uint8 out), 1 MiB / 0.5 MiB per DMA.
    qT_v = qT.rearrange("t b (c p) n -> t b p c n", p=P)
    out_v = out.rearrange("t b (c p) n -> t b p c n", p=P)
    kvT_v = kvT.rearrange("(c p) i -> p c i", p=P)
    wgT_v = wgT.rearrange("(c p) o -> p c o", p=P)

    with tile.TileContext(nc) as tc:
        with (
            tc.tile_pool(name="const", bufs=1) as const_pool,
            tc.tile_pool(name="qp", bufs=4) as q_pool,
            tc.tile_pool(name="wp", bufs=8) as w_pool,
            tc.tile_pool(name="mp", bufs=3) as m_pool,
            tc.tile_pool(name="a0p", bufs=2) as a0_pool,
            tc.tile_pool(name="psg", bufs=1, space="PSUM") as psg_pool,
        ):
            # ---- gate computation (all tiny) ----
            kvT_sb = const_pool.tile([P, C * NI], FP32, tag="kvT")
            nc.sync.dma_start(kvT_sb[:].rearrange("p (c i) -> p c i", c=C), kvT_v)
            wgT_sb = const_pool.tile([P, C * D], FP32, tag="wgT")
            nc.sync.dma_start(wgT_sb[:].rearrange("p (c o) -> p c o", c=C), wgT_v)

            # sum over t' of kvT (free layout per chunk: i = t'*16 + (b*4+nkv))
            kv4 = kvT_sb[:].rearrange("p (c tp i) -> p c tp i", c=C, tp=T)
            t01 = const_pool.tile([P, C * NG], FP32, tag="t01")
            t23 = const_pool.tile([P, C * NG], FP32, tag="t23")
            kvs = const_pool.tile([P, C * NG], FP32, tag="kvs")
            t01v = t01[:].rearrange("p (c i) -> p c i", c=C)
            t23v = t23[:].rearrange("p (c i) -> p c i", c=C)
            kvs_v = kvs[:].rearrange("p (c i) -> p c i", c=C)
            nc.vector.tensor_tensor(t01v, kv4[:, :, 0, :], kv4[:, :, 1, :], Alu.add)
            nc.vector.tensor_tensor(t23v, kv4[:, :, 2, :], kv4[:, :, 3, :], Alu.add)
            nc.vector.tensor_tensor(kvs_v, t01v, t23v, Alu.add)

            # g_raw[o_p, (co, b, nkv)] = sum_d wgT[d, o] * kvs[d, (b, nkv)]
            wgv = wgT_sb[:].rearrange("p (c o) -> p c o", c=C)
            psum_g = psg_pool.tile([P, C * NG], FP32)
            pg = psum_g[:].rearrange("p (c i) -> p c i", c=C)
            for co in range(C):
                for dc in range(C):
                    nc.tensor.matmul(
                        pg[:, co, :],
                        wgv[:, dc, co * P:(co + 1) * P],
                        kvs_v[:, dc, :],
                        start=(dc == 0), stop=(dc == C - 1),
                    )
            g_raw = const_pool.tile([P, C * NG], FP32, tag="graw")
            nc.scalar.copy(g_raw[:], psum_g[:])

            # per-step scaled gates: gs[p, (t, c, b)] = g_raw[p, (c, b, t)] * S2[t]
            graw_v = g_raw[:].rearrange("p (c b n) -> p c b n", c=C, b=B_LOC)
            g_s = const_pool.tile([P, T * C * B_LOC], FP32, tag="gs")
            gs_v = g_s[:].rearrange("p (t c b) -> p t c b", t=T, c=C)
            for t in range(T):
                nc.vector.tensor_scalar(
                    gs_v[:, t, :, :], graw_v[:, :, :, t], float(S2[t]), None,
                    Alu.mult,
                )

            # sigmoid threshold biases, one [P,1] per step
            biases = []
            for t in range(T):
                bt = const_pool.tile([P, 1], FP32, tag=f"bias{t}")
                nc.vector.memset(bt[:], float((TH[t] - 0.5) * BIG))
                biases.append(bt)

            if timing_mode:
                junk = const_pool.tile([P, C * NQ], I16, tag="junk")
                nc.vector.memset(junk[:], 0)
                junk4 = junk[:].rearrange("p (c n) -> p c n", c=C)
                for t in range(T):
                    for b in range(B_LOC):
                        nc.sync.dma_start(qT_v[t, b], junk4)
                nc.sync.dma_start(dummy, wgT_sb[:, :16])  # satisfy external output

            import contextlib
            if timing_mode and repeats > 1:
                assert repeats % UNROLL == 0
                rep_ctx = tc.For_i(0, repeats // UNROLL, 1)
                inner_reps = UNROLL
            else:
                rep_ctx = contextlib.nullcontext()
                inner_reps = 1

            # ---- main loop ----
            with rep_ctx:
             for _inner in range(inner_reps):
              w_prev = [None] * B_LOC
              for t in range(T):
                for b in range(B_LOC):
                    qt = q_pool.tile([P, C * NQ], I16, tag="q")
                    q4 = qt[:].rearrange("p (c n) -> p c n", c=C)
                    if "noload" not in ablate:
                        nc.sync.dma_start(q4, qT_v[t, b])
                    if t == 0:
                        # a = g*q on ACT (Copy with per-partition scale), off
                        # the DVE critical path; separate output tile.
                        at = a0_pool.tile([P, C * NQ], I16, tag="a0")
                        a4 = at[:].rearrange("p (c n) -> p c n", c=C)
                        if "noact" not in ablate:
                            for c in range(C):
                                nc.scalar.activation(
                                    a4[:, c, :], q4[:, c, :],
                                    mybir.ActivationFunctionType.Copy,
                                    bias=0.0, scale=gs_v[:, 0, c, b:b + 1],
                                )
                    else:
                        at = qt
                        if "nodve" not in ablate:
                            # a = g*q + w, in place over q (stt, 1x but single
                            # rounding site; the pre-saturation sum is required)
                            w4 = w_prev[b][:].rearrange("p (c n) -> p c n", c=C)
                            for c in range(C):
                                nc.vector.scalar_tensor_tensor(
                                    q4[:, c, :], q4[:, c, :],
                                    gs_v[:, t, c, b:b + 1], w4[:, c, :],
                                    Alu.mult, Alu.add,
                                )
                    # m = (a_i < TH) as exact 0/1 int16 via saturated sigmoid
                    # (the NOT-spike: host inverts; int16 keeps the reset TT at 2x)
                    mt = m_pool.tile([P, C * NQ], I16, tag="m")
                    if "noact" not in ablate:
                        nc.scalar.activation(
                            mt[:], at[:], mybir.ActivationFunctionType.Sigmoid,
                            bias=biases[t][:], scale=-BIG,
                        )
                    if "nostore" not in ablate:
                        # SWDGE store casts int16 {0,1} -> uint8 bytes
                        nc.gpsimd.dma_start(
                            out_v[t, b], mt[:].rearrange("p (c n) -> p c n", c=C))
                    if "nodve" not in ablate and t < T - 1:
                        # w = a * m  (hard reset; tensor_tensor, 2x int16)
                        wt = w_pool.tile([P, C * NQ], I16, tag="w")
                        nc.vector.tensor_tensor(wt[:], at[:], mt[:], Alu.mult)
                        w_prev[b] = wt
    nc.compile()
    return nc


_CACHED_NC = None


def _make_in_maps(q, kv, Wg):
    q = np.ascontiguousarray(q, dtype=np.float32)
    kv = np.ascontiguousarray(kv, dtype=np.float32)
    Wg = np.ascontiguousarray(Wg, dtype=np.float32)

    # transpose so d lands on partitions; quantize q to int16 fixed point
    qT = np.transpose(q, (0, 1, 3, 2))  # [T, B, D, NQ]
    q_i = np.clip(np.rint(qT * np.float32(S1)), -32767, 32767).astype(np.int16)
    wgT = (np.ascontiguousarray(Wg.T) * np.float32(1.0 / (2.0 * T * S1)))
    wgT = np.ascontiguousarray(wgT, dtype=np.float32)

    in_maps = []
    for i in range(N_CORES):
        b0 = i * B_LOC
        q_c = np.ascontiguousarray(q_i[:, b0:b0 + B_LOC])
        kv_i = kv[:, b0:b0 + B_LOC]  # [T, B_LOC, NKV, D]
        kvT_i = np.ascontiguousarray(
            kv_i.transpose(3, 0, 1, 2).reshape(D, T * B_LOC * NKV)
        )
        in_maps.append({"qT": q_c, "kvT": kvT_i, "wgT": wgT})
    return in_maps


def kernel(q: np.ndarray, kv: np.ndarray, Wg: np.ndarray) -> np.ndarray:
    global _CACHED_NC
    if _CACHED_NC is None:
        _CACHED_NC = build_kernel()
    nc = _CACHED_NC

    in_maps = _make_in_maps(q, kv, Wg)
    res = run_bass_kernel_spmd(nc, in_maps, core_ids=list(range(N_CORES)))
    m_u8 = np.concatenate([r["out"] for r in res.results], axis=1)  # [T,B,D,NQ]
    spikes = (m_u8 == 0)  # device emits NOT-spike
    return np.ascontiguousarray(spikes.transpose(0, 1, 3, 2)).astype(np.float32)


if __name__ == "__main__":
    rng = np.random.default_rng(0)
    q = rng.standard_normal((T, B, NQ, D), dtype=np.float32)
    kv = rng.standard_normal((T, B, NKV, D), dtype=np.float32)
    Wg = (rng.standard_normal((D, D), dtype=np.float32) / np.sqrt(D)).astype(np.float32)
    o = kernel(q, kv, Wg)
    print("out", o.shape, o.dtype, "mean", o.mean())


# revision 8
# speedup vs baseline: 1.3778x; 1.3778x over previous
"""Trainium2 Bass kernel for nn_MemoryUpdate (gated LIF memory update), v2.

Reference computation (fp32):
    k         = einsum('tbnd,od->tbno', kv, Wg)          # kv @ Wg^T
    gate_mean = mean_t'( k[t', b, nkv, d] )              # [Nkv, B, 1, D], Nkv == T
    update    = gate_mean[t, b, d] * q[t, b, n, d]       # broadcast over n
    spikes    = LIF over t: v' = (v + u)/2 ; s = v' >= 0.5 ; v = v' * (1 - s)

Shapes: q [4, 32, 1024, 512], kv [4, 32, 4, 512], Wg [512, 512] -> out [4, 32, 1024, 512].

v2 strategy (v1 was ~238 us, near the fp32 HBM roofline of ~187 us/core):
cut HBM traffic 2.7x and unlock 16-bit DVE perf modes.
  - d on partitions: host transposes q to [T, B, D, NQ]. The gate becomes a
    per-partition scalar, so the gate multiply fuses into the recurrence as
    scalar_tensor_tensor((q * g[P,1]) + w) — no broadcast tiles, no gate DMA.
  - q quantized host-side to int16 (scale 6000, max|q*S1| < 32767): halves
    the dominant read. Spikes written as uint8 (exact 0/1): quarter write.
    Per-core traffic 33.5+33.5 MB -> 16.8+8.4 MB.
  - int16 state with a doubling scale schedule S2[t] = 8192 << t: w_i stored
    at scale S2 *is* v/2 at scale 2*S2 = S2[t+1], so the LIF halving is free,
    each step has exactly one int16 rounding site (round-half-even, saturating
    — HW-verified), and precision doubles every step. Saturation at +/-32767
    only clips |v'| > 4 early (none exist) and |v'| > 2/1 late, where positive
    clips can't change the spike bit and negative clips are rare tails.
  - t=3 uses virtual scale 65536 via threshold 32767 (only s is needed).
  - spike threshold: ACT sigmoid(1e30 * a_i + (0.5 - TH)*1e30) -> uint8.
    Integer gap of 1 around TH makes the fma sign exact; sigmoid saturates
    to exact 0.0/1.0 (HW-verified probe).
Emulated end-to-end flip count vs fp32 reference: 463 of 67.1M outputs
(rel err 1.46e-2 < 2e-2 gate), matching HW cast semantics bit-for-bit.

Engine budget per core: DVE ~60 us (merged stt 2x_1P int16, w-op stt 2x,
t0 tensor_scalar 4x), ACT ~59 us (16 sigmoids [128,4096]), DMA ~70 us
(25.2 MB at ~355 GB/s). Target wall ~80-90 us.
"""

import sys

for p in ("/opt/trn_rl_repo", "/root/.axon_site/_ro/trn_rl_repo"):
    if p not in sys.path:
        sys.path.insert(0, p)

import numpy as np

import concourse.bass as bass
import concourse.mybir as mybir
import concourse.tile as tile
from concourse import bacc
from concourse.bass_utils import run_bass_kernel_spmd

# Problem constants (hardcoded per harness contract)
T, B, NQ, NKV, D = 4, 32, 1024, 4, 512
N_CORES = 8
B_LOC = B // N_CORES  # 4
P = 128               # partitions
C = D // P            # 4 d-chunks
NI = T * B_LOC * NKV  # 64 kvT columns
NG = B_LOC * NKV      # 16 gate rows per core

S1 = 6000.0                                    # q fixed-point scale
S2 = [8192.0, 16384.0, 32768.0, 65536.0]       # state scale per step (doubling)
TH = [4096.0, 8192.0, 16384.0, 32767.0]        # spike threshold in a_i units
BIG = 1.0e30

FP32 = mybir.dt.float32
I16 = mybir.dt.int16
U8 = mybir.dt.uint8
Alu = mybir.AluOpType
UNROLL = 4  # static inner unroll inside the timing-mode For_i loop


def build_kernel(repeats=1, timing_mode=False, ablate=frozenset()):
    nc = bacc.Bacc("TRN2", target_bir_lowering=False, debug=False,
                   num_devices=N_CORES)

    if timing_mode:
        # timing-only variant: big tensors live in internal DRAM so the wall
        # clock isn't dominated by host<->device transfers; the main body runs
        # `repeats` times in an on-device loop.
        qT = nc.dram_tensor("q_int", [T, B_LOC, D, NQ], I16).ap()
        out = nc.dram_tensor("out_int", [T, B_LOC, D, NQ], U8).ap()
        dummy = nc.dram_tensor("tiny_out", [P, 16], FP32, kind="ExternalOutput").ap()
    else:
        qT = nc.dram_tensor("qT", [T, B_LOC, D, NQ], I16, kind="ExternalInput").ap()
        out = nc.dram_tensor("out", [T, B_LOC, D, NQ], U8, kind="ExternalOutput").ap()
        dummy = None
    kvT = nc.dram_tensor("kvT", [D, NI], FP32, kind="ExternalInput").ap()
    # wgT is Wg^T host-scaled by 1/(2*T*S1): folds the t'-mean, the LIF /tau
    # and the int16 q descale into the gate.
    wgT = nc.dram_tensor("wgT", [D, D], FP32, kind="ExternalInput").ap()

    # d = c*128 + p on partitions; free = (c, n) -> per-partition runs of
    # 2 KiB (int16 q) / 1 KiB (uint8 out), 1 MiB / 0.5 MiB per DMA.
    qT_v = qT.rearrange("t b (c p) n -> t b p c n", p=P)
    out_v = out.rearrange("t b (c p) n -> t b p c n", p=P)
    kvT_v = kvT.rearrange("(c p) i -> p c i", p=P)
    wgT_v = wgT.rearrange("(c p) o -> p c o", p=P)

    with tile.TileContext(nc) as tc:
        with (
            tc.tile_pool(name="const", bufs=1) as const_pool,
            tc.tile_pool(name="qp", bufs=4) as q_pool,
            tc.tile_pool(name="wp", bufs=8) as w_pool,
            tc.tile_pool(name="mp", bufs=3) as m_pool,
            tc.tile_pool(name="a0p", bufs=2) as a0_pool,
            tc.tile_pool(name="psg", bufs=1, space="PSUM") as psg_pool,
        ):
            # ---- gate computation (all tiny) ----
            kvT_sb = const_pool.tile([P, C * NI], FP32, tag="kvT")
            nc.sync.dma_start(kvT_sb[:].rearrange("p (c i) -> p c i", c=C), kvT_v)
            wgT_sb = const_pool.tile([P, C * D], FP32, tag="wgT")
            nc.sync.dma_start(wgT_sb[:].rearrange("p (c o) -> p c o", c=C), wgT_v)

            # sum over t' of kvT (free layout per chunk: i = t'*16 + (b*4+nkv))
            kv4 = kvT_sb[:].rearrange("p (c tp i) -> p c tp i", c=C, tp=T)
            t01 = const_pool.tile([P, C * NG], FP32, tag="t01")
            t23 = const_pool.tile([P, C * NG], FP32, tag="t23")
            kvs = const_pool.tile([P, C * NG], FP32, tag="kvs")
            t01v = t01[:].rearrange("p (c i) -> p c i", c=C)
            t23v = t23[:].rearrange("p (c i) -> p c i", c=C)
            kvs_v = kvs[:].rearrange("p (c i) -> p c i", c=C)
            nc.vector.tensor_tensor(t01v, kv4[:, :, 0, :], kv4[:, :, 1, :], Alu.add)
            nc.vector.tensor_tensor(t23v, kv4[:, :, 2, :], kv4[:, :, 3, :], Alu.add)
            nc.vector.tensor_tensor(kvs_v, t01v, t23v, Alu.add)

            # g_raw[o_p, (co, b, nkv)] = sum_d wgT[d, o] * kvs[d, (b, nkv)]
            wgv = wgT_sb[:].rearrange("p (c o) -> p c o", c=C)
            psum_g = psg_pool.tile([P, C * NG], FP32)
            pg = psum_g[:].rearrange("p (c i) -> p c i", c=C)
            for co in range(C):
                for dc in range(C):
                    nc.tensor.matmul(
                        pg[:, co, :],
                        wgv[:, dc, co * P:(co + 1) * P],
                        kvs_v[:, dc, :],
                        start=(dc == 0), stop=(dc == C - 1),
                    )
            g_raw = const_pool.tile([P, C * NG], FP32, tag="graw")
            nc.scalar.copy(g_raw[:], psum_g[:])

            # per-step scaled gates: gs[p, (t, c, b)] = g_raw[p, (c, b, t)] * S2[t]
            graw_v = g_raw[:].rearrange("p (c b n) -> p c b n", c=C, b=B_LOC)
            g_s = const_pool.tile([P, T * C * B_LOC], FP32, tag="gs")
            gs_v = g_s[:].rearrange("p (t c b) -> p t c b", t=T, c=C)
            for t in range(T):
                nc.vector.tensor_scalar(
                    gs_v[:, t, :, :], graw_v[:, :, :, t], float(S2[t]), None,
                    Alu.mult,
                )

            # sigmoid threshold biases, one [P,1] per step
            biases = []
            for t in range(T):
                bt = const_pool.tile([P, 1], FP32, tag=f"bias{t}")
                nc.vector.memset(bt[:], float((TH[t] - 0.5) * BIG))
                biases.append(bt)

            if timing_mode:
                junk = const_pool.tile([P, C * NQ], I16, tag="junk")
                nc.vector.memset(junk[:], 0)
                junk4 = junk[:].rearrange("p (c n) -> p c n", c=C)
                for t in range(T):
                    for b in range(B_LOC):
                        nc.sync.dma_start(qT_v[t, b], junk4)
                nc.sync.dma_start(dummy, wgT_sb[:, :16])  # satisfy external output

            import contextlib
            if timing_mode and repeats > 1:
                assert repeats % UNROLL == 0
                rep_ctx = tc.For_i(0, repeats // UNROLL, 1)
                inner_reps = UNROLL
            else:
                rep_ctx = contextlib.nullcontext()
                inner_reps = 1

            # ---- main loop ----
            with rep_ctx:
             for _inner in range(inner_reps):
              w_prev = [None] * B_LOC
              pending_w = None  # deferred reset TT: emitted after the next
                                # tile's stt group so DVE never waits on ACT
              for t in range(T):
                for b in range(B_LOC):
                    qt = q_pool.tile([P, C * NQ], I16, tag="q")
                    q4 = qt[:].rearrange("p (c n) -> p c n", c=C)
                    if "noload" not in ablate:
                        nc.sync.dma_start(q4, qT_v[t, b])
                    if t == 0:
                        # a = g*q on ACT (Copy with per-partition scale), off
                        # the DVE critical path; separate output tile.
                        at = a0_pool.tile([P, C * NQ], I16, tag="a0")
                        a4 = at[:].rearrange("p (c n) -> p c n", c=C)
                        if "noact" not in ablate:
                            for c in range(C):
                                nc.scalar.activation(
                                    a4[:, c, :], q4[:, c, :],
                                    mybir.ActivationFunctionType.Copy,
                                    bias=0.0, scale=gs_v[:, 0, c, b:b + 1],
                                )
                    else:
                        at = qt
                        if "nodve" not in ablate:
                            # a = g*q + w, in place over q (stt, 1x but single
                            # rounding site; the pre-saturation sum is required)
                            w4 = w_prev[b][:].rearrange("p (c n) -> p c n", c=C)
                            for c in range(C):
                                nc.vector.scalar_tensor_tensor(
                                    q4[:, c, :], q4[:, c, :],
                                    gs_v[:, t, c, b:b + 1], w4[:, c, :],
                                    Alu.mult, Alu.add,
                                )
                    if pending_w is not None:
                        pending_w()
                        pending_w = None
                    # m = (a_i < TH) as exact 0/1 int16 via saturated sigmoid
                    # (the NOT-spike: host inverts; int16 keeps the reset TT at 2x)
                    mt = m_pool.tile([P, C * NQ], I16, tag="m")
                    if "noact" not in ablate:
                        nc.scalar.activation(
                            mt[:], at[:], mybir.ActivationFunctionType.Sigmoid,
                            bias=biases[t][:], scale=-BIG,
                        )
                    if "nostore" not in ablate:
                        # SWDGE store casts int16 {0,1} -> uint8 bytes
                        nc.gpsimd.dma_start(
                            out_v[t, b], mt[:].rearrange("p (c n) -> p c n", c=C))
                    if "nodve" not in ablate and t < T - 1:
                        # w = a * m  (hard reset; tensor_tensor, 2x int16)
                        wt = w_pool.tile([P, C * NQ], I16, tag="w")

                        def emit_w(wt=wt, at=at, mt=mt):
                            nc.vector.tensor_tensor(wt[:], at[:], mt[:], Alu.mult)

                        pending_w = emit_w
                        w_prev[b] = wt
              if pending_w is not None:
                  pending_w()
                  pending_w = None
    nc.compile()
    return nc


_CACHED_NC = None


def _make_in_maps(q, kv, Wg):
    q = np.ascontiguousarray(q, dtype=np.float32)
    kv = np.ascontiguousarray(kv, dtype=np.float32)
    Wg = np.ascontiguousarray(Wg, dtype=np.float32)

    # transpose so d lands on partitions; quantize q to int16 fixed point
    qT = np.transpose(q, (0, 1, 3, 2))  # [T, B, D, NQ]
    q_i = np.clip(np.rint(qT * np.float32(S1)), -32767, 32767).astype(np.int16)
    wgT = (np.ascontiguousarray(Wg.T) * np.float32(1.0 / (2.0 * T * S1)))
    wgT = np.ascontiguousarray(wgT, dtype=np.float32)

    in_maps = []
    for i in range(N_CORES):
        b0 = i * B_LOC
        q_c = np.ascontiguousarray(q_i[:, b0:b0 + B_LOC])
        kv_i = kv[:, b0:b0 + B_LOC]  # [T, B_LOC, NKV, D]
        kvT_i = np.ascontiguousarray(
            kv_i.transpose(3, 0, 1, 2).reshape(D, T * B_LOC * NKV)
        )
        in_maps.append({"qT": q_c, "kvT": kvT_i, "wgT": wgT})
    return in_maps


def kernel(q: np.ndarray, kv: np.ndarray, Wg: np.ndarray) -> np.ndarray:
    global _CACHED_NC
    if _CACHED_NC is None:
        _CACHED_NC = build_kernel()
    nc = _CACHED_NC

    in_maps = _make_in_maps(q, kv, Wg)
    res = run_bass_kernel_spmd(nc, in_maps, core_ids=list(range(N_CORES)))
    m_u8 = np.concatenate([r["out"] for r in res.results], axis=1)  # [T,B,D,NQ]
    spikes = (m_u8 == 0)  # device emits NOT-spike
    return np.ascontiguousarray(spikes.transpose(0, 1, 3, 2)).astype(np.float32)


if __name__ == "__main__":
    rng = np.random.default_rng(0)
    q = rng.standard_normal((T, B, NQ, D), dtype=np.float32)
    kv = rng.standard_normal((T, B, NKV, D), dtype=np.float32)
    Wg = (rng.standard_normal((D, D), dtype=np.float32) / np.sqrt(D)).astype(np.float32)
    o = kernel(q, kv, Wg)
    print("out", o.shape, o.dtype, "mean", o.mean())


# revision 9
# speedup vs baseline: 1.4359x; 1.0422x over previous
"""Trainium2 Bass kernel for nn_MemoryUpdate (gated LIF memory update), v2.

Reference computation (fp32):
    k         = einsum('tbnd,od->tbno', kv, Wg)          # kv @ Wg^T
    gate_mean = mean_t'( k[t', b, nkv, d] )              # [Nkv, B, 1, D], Nkv == T
    update    = gate_mean[t, b, d] * q[t, b, n, d]       # broadcast over n
    spikes    = LIF over t: v' = (v + u)/2 ; s = v' >= 0.5 ; v = v' * (1 - s)

Shapes: q [4, 32, 1024, 512], kv [4, 32, 4, 512], Wg [512, 512] -> out [4, 32, 1024, 512].

v2 strategy (v1 was ~238 us, near the fp32 HBM roofline of ~187 us/core):
cut HBM traffic 2.7x and unlock 16-bit DVE perf modes.
  - d on partitions: host transposes q to [T, B, D, NQ]. The gate becomes a
    per-partition scalar, so the gate multiply fuses into the recurrence as
    scalar_tensor_tensor((q * g[P,1]) + w) — no broadcast tiles, no gate DMA.
  - q quantized host-side to int16 (scale 6000, max|q*S1| < 32767): halves
    the dominant read. Spikes written as uint8 (exact 0/1): quarter write.
    Per-core traffic 33.5+33.5 MB -> 16.8+8.4 MB.
  - int16 state with a doubling scale schedule S2[t] = 8192 << t: w_i stored
    at scale S2 *is* v/2 at scale 2*S2 = S2[t+1], so the LIF halving is free,
    each step has exactly one int16 rounding site (round-half-even, saturating
    — HW-verified), and precision doubles every step. Saturation at +/-32767
    only clips |v'| > 4 early (none exist) and |v'| > 2/1 late, where positive
    clips can't change the spike bit and negative clips are rare tails.
  - t=3 uses virtual scale 65536 via threshold 32767 (only s is needed).
  - spike threshold: ACT sigmoid(1e30 * a_i + (0.5 - TH)*1e30) -> uint8.
    Integer gap of 1 around TH makes the fma sign exact; sigmoid saturates
    to exact 0.0/1.0 (HW-verified probe).
Emulated end-to-end flip count vs fp32 reference: 463 of 67.1M outputs
(rel err 1.46e-2 < 2e-2 gate), matching HW cast semantics bit-for-bit.

Engine budget per core: DVE ~60 us (merged stt 2x_1P int16, w-op stt 2x,
t0 tensor_scalar 4x), ACT ~59 us (16 sigmoids [128,4096]), DMA ~70 us
(25.2 MB at ~355 GB/s). Target wall ~80-90 us.
"""

import sys

for p in ("/opt/trn_rl_repo", "/root/.axon_site/_ro/trn_rl_repo"):
    if p not in sys.path:
        sys.path.insert(0, p)

import numpy as np

import concourse.bass as bass
import concourse.mybir as mybir
import concourse.tile as tile
from concourse import bacc
from concourse.bass_utils import run_bass_kernel_spmd

# Problem constants (hardcoded per harness contract)
T, B, NQ, NKV, D = 4, 32, 1024, 4, 512
N_CORES = 8
B_LOC = B // N_CORES  # 4
P = 128               # partitions
C = D // P            # 4 d-chunks
NI = T * B_LOC * NKV  # 64 kvT columns
NG = B_LOC * NKV      # 16 gate rows per core

S1 = 6000.0                                    # q fixed-point scale
S2 = [8192.0, 16384.0, 32768.0, 65536.0]       # state scale per step (doubling)
TH = [4096.0, 8192.0, 16384.0, 32767.0]        # spike threshold in a_i units
BIG = 1.0e30

FP32 = mybir.dt.float32
I16 = mybir.dt.int16
U8 = mybir.dt.uint8
Alu = mybir.AluOpType
UNROLL = 4  # static inner unroll inside the timing-mode For_i loop


def build_kernel(repeats=1, timing_mode=False, ablate=frozenset()):
    nc = bacc.Bacc("TRN2", target_bir_lowering=False, debug=False,
                   num_devices=N_CORES)

    if timing_mode:
        # timing-only variant: big tensors live in internal DRAM so the wall
        # clock isn't dominated by host<->device transfers; the main body runs
        # `repeats` times in an on-device loop.
        qT = nc.dram_tensor("q_int", [T, B_LOC, D, NQ], I16).ap()
        out = nc.dram_tensor("out_int", [T, B_LOC, D, NQ], U8).ap()
        dummy = nc.dram_tensor("tiny_out", [P, 16], FP32, kind="ExternalOutput").ap()
    else:
        qT = nc.dram_tensor("qT", [T, B_LOC, D, NQ], I16, kind="ExternalInput").ap()
        out = nc.dram_tensor("out", [T, B_LOC, D, NQ], U8, kind="ExternalOutput").ap()
        dummy = None
    kvT = nc.dram_tensor("kvT", [D, NI], FP32, kind="ExternalInput").ap()
    # wgT is Wg^T host-scaled by 1/(2*T*S1): folds the t'-mean, the LIF /tau
    # and the int16 q descale into the gate.
    wgT = nc.dram_tensor("wgT", [D, D], FP32, kind="ExternalInput").ap()

    # d = c*128 + p on partitions; free = (c, n) -> per-partition runs of
    # 2 KiB (int16 q) / 1 KiB (uint8 out), 1 MiB / 0.5 MiB per DMA.
    qT_v = qT.rearrange("t b (c p) n -> t b p c n", p=P)
    out_v = out.rearrange("t b (c p) n -> t b p c n", p=P)
    kvT_v = kvT.rearrange("(c p) i -> p c i", p=P)
    wgT_v = wgT.rearrange("(c p) o -> p c o", p=P)

    with tile.TileContext(nc) as tc:
        with (
            tc.tile_pool(name="const", bufs=1) as const_pool,
            tc.tile_pool(name="qp", bufs=4) as q_pool,
            tc.tile_pool(name="wp", bufs=8) as w_pool,
            tc.tile_pool(name="mp", bufs=3) as m_pool,
            tc.tile_pool(name="psg", bufs=1, space="PSUM") as psg_pool,
        ):
            # ---- gate computation (all tiny) ----
            kvT_sb = const_pool.tile([P, C * NI], FP32, tag="kvT")
            nc.sync.dma_start(kvT_sb[:].rearrange("p (c i) -> p c i", c=C), kvT_v)
            wgT_sb = const_pool.tile([P, C * D], FP32, tag="wgT")
            nc.sync.dma_start(wgT_sb[:].rearrange("p (c o) -> p c o", c=C), wgT_v)

            # sum over t' of kvT (free layout per chunk: i = t'*16 + (b*4+nkv))
            kv4 = kvT_sb[:].rearrange("p (c tp i) -> p c tp i", c=C, tp=T)
            t01 = const_pool.tile([P, C * NG], FP32, tag="t01")
            t23 = const_pool.tile([P, C * NG], FP32, tag="t23")
            kvs = const_pool.tile([P, C * NG], FP32, tag="kvs")
            t01v = t01[:].rearrange("p (c i) -> p c i", c=C)
            t23v = t23[:].rearrange("p (c i) -> p c i", c=C)
            kvs_v = kvs[:].rearrange("p (c i) -> p c i", c=C)
            nc.vector.tensor_tensor(t01v, kv4[:, :, 0, :], kv4[:, :, 1, :], Alu.add)
            nc.vector.tensor_tensor(t23v, kv4[:, :, 2, :], kv4[:, :, 3, :], Alu.add)
            nc.vector.tensor_tensor(kvs_v, t01v, t23v, Alu.add)

            # g_raw[o_p, (co, b, nkv)] = sum_d wgT[d, o] * kvs[d, (b, nkv)]
            wgv = wgT_sb[:].rearrange("p (c o) -> p c o", c=C)
            psum_g = psg_pool.tile([P, C * NG], FP32)
            pg = psum_g[:].rearrange("p (c i) -> p c i", c=C)
            for co in range(C):
                for dc in range(C):
                    nc.tensor.matmul(
                        pg[:, co, :],
                        wgv[:, dc, co * P:(co + 1) * P],
                        kvs_v[:, dc, :],
                        start=(dc == 0), stop=(dc == C - 1),
                    )
            g_raw = const_pool.tile([P, C * NG], FP32, tag="graw")
            nc.scalar.copy(g_raw[:], psum_g[:])

            # per-step scaled gates: gs[p, (t, c, b)] = g_raw[p, (c, b, t)] * S2[t]
            graw_v = g_raw[:].rearrange("p (c b n) -> p c b n", c=C, b=B_LOC)
            g_s = const_pool.tile([P, T * C * B_LOC], FP32, tag="gs")
            gs_v = g_s[:].rearrange("p (t c b) -> p t c b", t=T, c=C)
            for t in range(T):
                nc.vector.tensor_scalar(
                    gs_v[:, t, :, :], graw_v[:, :, :, t], float(S2[t]), None,
                    Alu.mult,
                )

            # sigmoid threshold biases, one [P,1] per step
            biases = []
            for t in range(T):
                bt = const_pool.tile([P, 1], FP32, tag=f"bias{t}")
                nc.vector.memset(bt[:], float((TH[t] - 0.5) * BIG))
                biases.append(bt)

            if timing_mode:
                junk = const_pool.tile([P, C * NQ], I16, tag="junk")
                nc.vector.memset(junk[:], 0)
                junk4 = junk[:].rearrange("p (c n) -> p c n", c=C)
                for t in range(T):
                    for b in range(B_LOC):
                        nc.sync.dma_start(qT_v[t, b], junk4)
                nc.sync.dma_start(dummy, wgT_sb[:, :16])  # satisfy external output

            import contextlib
            if timing_mode and repeats > 1:
                assert repeats % UNROLL == 0
                rep_ctx = tc.For_i(0, repeats // UNROLL, 1)
                inner_reps = UNROLL
            else:
                rep_ctx = contextlib.nullcontext()
                inner_reps = 1

            # ---- main loop ----
            with rep_ctx:
             for _inner in range(inner_reps):
              w_prev = [None] * B_LOC
              pending_w = None  # deferred reset TT: emitted after the next
                                # tile's stt group so DVE never waits on ACT
              for t in range(T):
                for b in range(B_LOC):
                    qt = q_pool.tile([P, C * NQ], I16, tag="q")
                    q4 = qt[:].rearrange("p (c n) -> p c n", c=C)
                    if "noload" not in ablate:
                        nc.sync.dma_start(q4, qT_v[t, b])
                    if t == 0:
                        # a = g*q in place (tensor_scalar, 4x int16): keeps the
                        # t0 phase DVE/ACT-balanced (ACT t0 burst would stall
                        # the in-order DVE b-chains)
                        at = qt
                        if "nodve" not in ablate:
                            for c in range(C):
                                nc.vector.tensor_scalar(
                                    q4[:, c, :], q4[:, c, :],
                                    gs_v[:, 0, c, b:b + 1], None, Alu.mult,
                                )
                    else:
                        at = qt
                        if "nodve" not in ablate:
                            # a = g*q + w, in place over q (stt, 1x but single
                            # rounding site; the pre-saturation sum is required)
                            w4 = w_prev[b][:].rearrange("p (c n) -> p c n", c=C)
                            for c in range(C):
                                nc.vector.scalar_tensor_tensor(
                                    q4[:, c, :], q4[:, c, :],
                                    gs_v[:, t, c, b:b + 1], w4[:, c, :],
                                    Alu.mult, Alu.add,
                                )
                    if pending_w is not None:
                        pending_w()
                        pending_w = None
                    # m = (a_i < TH) as exact 0/1 int16 via saturated sigmoid
                    # (the NOT-spike: host inverts; int16 keeps the reset TT at 2x)
                    mt = m_pool.tile([P, C * NQ], I16, tag="m")
                    if "noact" not in ablate:
                        nc.scalar.activation(
                            mt[:], at[:], mybir.ActivationFunctionType.Sigmoid,
                            bias=biases[t][:], scale=-BIG,
                        )
                    if "nostore" not in ablate:
                        # SWDGE store casts int16 {0,1} -> uint8 bytes
                        nc.gpsimd.dma_start(
                            out_v[t, b], mt[:].rearrange("p (c n) -> p c n", c=C))
                    if "nodve" not in ablate and t < T - 1:
                        # w = a * m  (hard reset; tensor_tensor, 2x int16)
                        wt = w_pool.tile([P, C * NQ], I16, tag="w")

                        def emit_w(wt=wt, at=at, mt=mt):
                            nc.vector.tensor_tensor(wt[:], at[:], mt[:], Alu.mult)

                        pending_w = emit_w
                        w_prev[b] = wt
              if pending_w is not None:
                  pending_w()
                  pending_w = None
    nc.compile()
    return nc


_CACHED_NC = None


def _make_in_maps(q, kv, Wg):
    q = np.ascontiguousarray(q, dtype=np.float32)
    kv = np.ascontiguousarray(kv, dtype=np.float32)
    Wg = np.ascontiguousarray(Wg, dtype=np.float32)

    # transpose so d lands on partitions; quantize q to int16 fixed point
    qT = np.transpose(q, (0, 1, 3, 2))  # [T, B, D, NQ]
    q_i = np.clip(np.rint(qT * np.float32(S1)), -32767, 32767).astype(np.int16)
    wgT = (np.ascontiguousarray(Wg.T) * np.float32(1.0 / (2.0 * T * S1)))
    wgT = np.ascontiguousarray(wgT, dtype=np.float32)

    in_maps = []
    for i in range(N_CORES):
        b0 = i * B_LOC
        q_c = np.ascontiguousarray(q_i[:, b0:b0 + B_LOC])
        kv_i = kv[:, b0:b0 + B_LOC]  # [T, B_LOC, NKV, D]
        kvT_i = np.ascontiguousarray(
            kv_i.transpose(3, 0, 1, 2).reshape(D, T * B_LOC * NKV)
        )
        in_maps.append({"qT": q_c, "kvT": kvT_i, "wgT": wgT})
    return in_maps


def kernel(q: np.ndarray, kv: np.ndarray, Wg: np.ndarray) -> np.ndarray:
    global _CACHED_NC
    if _CACHED_NC is None:
        _CACHED_NC = build_kernel()
    nc = _CACHED_NC

    in_maps = _make_in_maps(q, kv, Wg)
    res = run_bass_kernel_spmd(nc, in_maps, core_ids=list(range(N_CORES)))
    m_u8 = np.concatenate([r["out"] for r in res.results], axis=1)  # [T,B,D,NQ]
    spikes = (m_u8 == 0)  # device emits NOT-spike
    return np.ascontiguousarray(spikes.transpose(0, 1, 3, 2)).astype(np.float32)


if __name__ == "__main__":
    rng = np.random.default_rng(0)
    q = rng.standard_normal((T, B, NQ, D), dtype=np.float32)
    kv = rng.standard_normal((T, B, NKV, D), dtype=np.float32)
    Wg = (rng.standard_normal((D, D), dtype=np.float32) / np.sqrt(D)).astype(np.float32)
    o = kernel(q, kv, Wg)
    print("out", o.shape, o.dtype, "mean", o.mean())


# revision 17
# speedup vs baseline: 1.5691x; 1.0928x over previous
"""Trainium2 Bass kernel for nn_MemoryUpdate (gated LIF memory update), v3.

Reference computation (fp32):
    k         = einsum('tbnd,od->tbno', kv, Wg)          # kv @ Wg^T
    gate_mean = mean_t'( k[t', b, nkv, d] )              # [Nkv, B, 1, D], Nkv == T
    update    = gate_mean[t, b, d] * q[t, b, n, d]       # broadcast over n
    spikes    = LIF over t: v' = (v + u)/2 ; s = v' >= 0.5 ; v = v' * (1 - s)

Shapes: q [4, 32, 1024, 512], kv [4, 32, 4, 512], Wg [512, 512] -> out [4, 32, 1024, 512].

v3 strategy (v1 was ~238 us, near the fp32 HBM roofline of ~187 us/core;
v3 measures ~96 us): cut HBM traffic 2.7x, use 16-bit DVE modes, keep every
engine's critical chain short.
  - d on partitions: host transposes q to [T, B, D, NQ]. The gate becomes a
    per-partition scalar, so the gate multiply fuses into the recurrence as
    scalar_tensor_tensor((q * g[P,1]) + w) — no broadcast tiles, no gate DMA.
  - q quantized host-side to int16 (scale 6000, max|q*S1| < 32767): halves
    the dominant read. Output written as 1 byte/elem: per-core traffic
    33.5+33.5 MB -> 16.8+8.4 MB (dma-only ablation: 75.6 us).
  - int16 state with a doubling scale schedule S2[t] = 8192 << t: w_i stored
    at scale S2 *is* v/2 at scale 2*S2 = S2[t+1], so the LIF halving is free,
    each step has exactly one int16 rounding site (round-half-even, saturating
    — HW-verified), and precision doubles every step. The merged stt must sum
    g*q + w BEFORE the saturating cast (split u/a saturates u alone: 191K
    flips). Positive clips can't change the spike bit; negative clips are
    rare tails. t=3 uses virtual scale 65536 via threshold 32767.
  - ACT emits m = NOT-spike = (a_i < TH) as int16 {0,1} via saturated
    sigmoid(-1e30*a_i + (TH-0.5)*1e30) — exact (integer gap + true fma,
    HW-verified). int16 m keeps the reset multiply w = a*m on the DVE
    tensor_tensor 2x_1P path (stt is 1x-only: measured 4523 ns vs TT 2327).
    The SWDGE store casts m to uint8 bytes; the HOST inverts (s = m == 0).
  - per-op rates (HW-measured, [128,4096] int16): ts 4x 1.2us, TT 2x 2.3us,
    stt 1x 4.5us, ACT 3.7us. Per core: DVE ~92 us (48 stt-chunk merged ops
    58 + 12 TT resets 26 + 16 t0 ts 6), ACT ~59 us, DMA ~76 us. The reset TT
    is software-pipelined one tile behind its ACT m-pass (in-order engines).
Flip count vs fp32 reference: 463 of 67.1M outputs (rel 1.46e-2 < 2e-2 gate),
bit-identical to the numpy emulation of the int16 pipeline.
"""

import sys

for p in ("/opt/trn_rl_repo", "/root/.axon_site/_ro/trn_rl_repo"):
    if p not in sys.path:
        sys.path.insert(0, p)

import numpy as np

import concourse.bass as bass
import concourse.mybir as mybir
import concourse.tile as tile
from concourse import bacc
from concourse.bass_utils import run_bass_kernel_spmd

# Problem constants (hardcoded per harness contract)
T, B, NQ, NKV, D = 4, 32, 1024, 4, 512
N_CORES = 8
B_LOC = B // N_CORES  # 4
P = 128               # partitions
C = D // P            # 4 d-chunks
NI = T * B_LOC * NKV  # 64 kvT columns
NG = B_LOC * NKV      # 16 gate rows per core

S1 = 6000.0                                    # q fixed-point scale
S2 = [8192.0, 16384.0, 32768.0, 65536.0]       # state scale per step (doubling)
TH = [4096.0, 8192.0, 16384.0, 32767.0]        # spike threshold in a_i units
BIG = 1.0e30

FP32 = mybir.dt.float32
I16 = mybir.dt.int16
U8 = mybir.dt.uint8
Alu = mybir.AluOpType
UNROLL = 4  # static inner unroll inside the timing-mode For_i loop


def build_kernel(repeats=1, timing_mode=False, ablate=frozenset()):
    nc = bacc.Bacc("TRN2", target_bir_lowering=False, debug=False,
                   num_devices=N_CORES)

    if timing_mode:
        # timing-only variant: big tensors live in internal DRAM so the wall
        # clock isn't dominated by host<->device transfers; the main body runs
        # `repeats` times in an on-device loop.
        qT = nc.dram_tensor("q_int", [T, B_LOC, D, NQ], I16).ap()
        out = nc.dram_tensor("out_int", [T, B_LOC, D, NQ], U8).ap()
        dummy = nc.dram_tensor("tiny_out", [P, 16], FP32, kind="ExternalOutput").ap()
    else:
        qT = nc.dram_tensor("qT", [T, B_LOC, D, NQ], I16, kind="ExternalInput").ap()
        out = nc.dram_tensor("out", [T, B_LOC, D, NQ], U8, kind="ExternalOutput").ap()
        dummy = None
    kvT = nc.dram_tensor("kvT", [D, NI], FP32, kind="ExternalInput").ap()
    # wgT is Wg^T host-scaled by 1/(2*T*S1): folds the t'-mean, the LIF /tau
    # and the int16 q descale into the gate.
    wgT = nc.dram_tensor("wgT", [D, D], FP32, kind="ExternalInput").ap()

    # d = c*128 + p on partitions; free = (c, n) -> per-partition runs of
    # 2 KiB (int16 q) / 1 KiB (uint8 out), 1 MiB / 0.5 MiB per DMA.
    qT_v = qT.rearrange("t b (c p) n -> t b p c n", p=P)
    out_v = out.rearrange("t b (c p) n -> t b p c n", p=P)
    kvT_v = kvT.rearrange("(c p) i -> p c i", p=P)
    wgT_v = wgT.rearrange("(c p) o -> p c o", p=P)

    with tile.TileContext(nc) as tc:
        with (
            tc.tile_pool(name="const", bufs=1) as const_pool,
            tc.tile_pool(name="qp", bufs=8) as q_pool,
            tc.tile_pool(name="wp", bufs=8) as w_pool,
            tc.tile_pool(name="mp", bufs=6) as m_pool,
            tc.tile_pool(name="psg", bufs=1, space="PSUM") as psg_pool,
        ):
            # ---- gate computation (all tiny) ----
            kvT_sb = const_pool.tile([P, C * NI], FP32, tag="kvT")
            nc.sync.dma_start(kvT_sb[:].rearrange("p (c i) -> p c i", c=C), kvT_v)
            wgT_sb = const_pool.tile([P, C * D], FP32, tag="wgT")
            nc.sync.dma_start(wgT_sb[:].rearrange("p (c o) -> p c o", c=C), wgT_v)

            # sum over t' of kvT (free layout per chunk: i = t'*16 + (b*4+nkv))
            kv4 = kvT_sb[:].rearrange("p (c tp i) -> p c tp i", c=C, tp=T)
            t01 = const_pool.tile([P, C * NG], FP32, tag="t01")
            t23 = const_pool.tile([P, C * NG], FP32, tag="t23")
            kvs = const_pool.tile([P, C * NG], FP32, tag="kvs")
            t01v = t01[:].rearrange("p (c i) -> p c i", c=C)
            t23v = t23[:].rearrange("p (c i) -> p c i", c=C)
            kvs_v = kvs[:].rearrange("p (c i) -> p c i", c=C)
            nc.vector.tensor_tensor(t01v, kv4[:, :, 0, :], kv4[:, :, 1, :], Alu.add)
            nc.vector.tensor_tensor(t23v, kv4[:, :, 2, :], kv4[:, :, 3, :], Alu.add)
            nc.vector.tensor_tensor(kvs_v, t01v, t23v, Alu.add)

            # g_raw[o_p, (co, b, nkv)] = sum_d wgT[d, o] * kvs[d, (b, nkv)]
            wgv = wgT_sb[:].rearrange("p (c o) -> p c o", c=C)
            psum_g = psg_pool.tile([P, C * NG], FP32)
            pg = psum_g[:].rearrange("p (c i) -> p c i", c=C)
            for co in range(C):
                for dc in range(C):
                    nc.tensor.matmul(
                        pg[:, co, :],
                        wgv[:, dc, co * P:(co + 1) * P],
                        kvs_v[:, dc, :],
                        start=(dc == 0), stop=(dc == C - 1),
                    )
            g_raw = const_pool.tile([P, C * NG], FP32, tag="graw")
            nc.scalar.copy(g_raw[:], psum_g[:])

            # per-step scaled gates: gs[p, (t, c, b)] = g_raw[p, (c, b, t)] * S2[t]
            graw_v = g_raw[:].rearrange("p (c b n) -> p c b n", c=C, b=B_LOC)
            g_s = const_pool.tile([P, T * C * B_LOC], FP32, tag="gs")
            gs_v = g_s[:].rearrange("p (t c b) -> p t c b", t=T, c=C)
            for t in range(T):
                nc.vector.tensor_scalar(
                    gs_v[:, t, :, :], graw_v[:, :, :, t], float(S2[t]), None,
                    Alu.mult,
                )

            # sigmoid threshold biases, one [P,1] per step
            biases = []
            for t in range(T):
                bt = const_pool.tile([P, 1], FP32, tag=f"bias{t}")
                nc.vector.memset(bt[:], float((TH[t] - 0.5) * BIG))
                biases.append(bt)

            if timing_mode:
                junk = const_pool.tile([P, C * NQ], I16, tag="junk")
                nc.vector.memset(junk[:], 0)
                junk4 = junk[:].rearrange("p (c n) -> p c n", c=C)
                for t in range(T):
                    for b in range(B_LOC):
                        nc.sync.dma_start(qT_v[t, b], junk4)
                nc.sync.dma_start(dummy, wgT_sb[:, :16])  # satisfy external output

            import contextlib
            if timing_mode and repeats > 1:
                assert repeats % UNROLL == 0
                rep_ctx = tc.For_i(0, repeats // UNROLL, 1)
                inner_reps = UNROLL
            else:
                rep_ctx = contextlib.nullcontext()
                inner_reps = 1

            # ---- main loop ----
            with rep_ctx:
             for _inner in range(inner_reps):
              w_prev = [None] * B_LOC
              pending_w = None  # deferred reset TT: emitted after the next
                                # tile's stt group so DVE never waits on ACT
              for t in range(T):
                for b in range(B_LOC):
                    qt = q_pool.tile([P, C * NQ], I16, tag="q")
                    q4 = qt[:].rearrange("p (c n) -> p c n", c=C)
                    if "noload" not in ablate:
                        nc.sync.dma_start(q4, qT_v[t, b])
                    if t == 0:
                        # a = g*q in place (tensor_scalar, 4x int16). Keeping
                        # t0 on DVE beats ACT here: an ACT t0 burst serializes
                        # the in-order per-b chains (measured 98 vs 91 us).
                        at = qt
                        if "nodve" not in ablate:
                            for c in range(C):
                                nc.vector.tensor_scalar(
                                    q4[:, c, :], q4[:, c, :],
                                    gs_v[:, 0, c, b:b + 1], None, Alu.mult,
                                )
                    else:
                        at = qt
                        if "nodve" not in ablate:
                            # a = g*q + w, in place over q (stt, 1x but single
                            # rounding site; the pre-saturation sum is required)
                            w4 = w_prev[b][:].rearrange("p (c n) -> p c n", c=C)
                            for c in range(C):
                                nc.vector.scalar_tensor_tensor(
                                    q4[:, c, :], q4[:, c, :],
                                    gs_v[:, t, c, b:b + 1], w4[:, c, :],
                                    Alu.mult, Alu.add,
                                )
                    if pending_w is not None:
                        pending_w()
                        pending_w = None
                    # m = (a_i < TH) as exact 0/1 int16 via saturated sigmoid
                    # (the NOT-spike: host inverts; int16 keeps the reset TT at 2x)
                    mt = m_pool.tile([P, C * NQ], I16, tag="m")
                    if "noact" not in ablate:
                        nc.scalar.activation(
                            mt[:], at[:], mybir.ActivationFunctionType.Sigmoid,
                            bias=biases[t][:], scale=-BIG,
                        )
                    if "nostore" not in ablate:
                        # SWDGE store casts int16 {0,1} -> uint8 bytes
                        src_t = junk if ("noact" in ablate and timing_mode) else mt
                        nc.gpsimd.dma_start(
                            out_v[t, b], src_t[:].rearrange("p (c n) -> p c n", c=C))
                    if "nodve" not in ablate and t < T - 1:
                        # w = a * m  (hard reset; tensor_tensor, 2x int16)
                        wt = w_pool.tile([P, C * NQ], I16, tag="w")

                        def emit_w(wt=wt, at=at, mt=mt):
                            nc.vector.tensor_tensor(wt[:], at[:], mt[:], Alu.mult)

                        pending_w = emit_w
                        w_prev[b] = wt
              if pending_w is not None:
                  pending_w()
                  pending_w = None
    nc.compile()
    return nc


_CACHED_NC = None


def _make_in_maps(q, kv, Wg):
    q = np.ascontiguousarray(q, dtype=np.float32)
    kv = np.ascontiguousarray(kv, dtype=np.float32)
    Wg = np.ascontiguousarray(Wg, dtype=np.float32)

    # transpose so d lands on partitions; quantize q to int16 fixed point
    qT = np.transpose(q, (0, 1, 3, 2))  # [T, B, D, NQ]
    q_i = np.clip(np.rint(qT * np.float32(S1)), -32767, 32767).astype(np.int16)
    wgT = (np.ascontiguousarray(Wg.T) * np.float32(1.0 / (2.0 * T * S1)))
    wgT = np.ascontiguousarray(wgT, dtype=np.float32)

    in_maps = []
    for i in range(N_CORES):
        b0 = i * B_LOC
        q_c = np.ascontiguousarray(q_i[:, b0:b0 + B_LOC])
        kv_i = kv[:, b0:b0 + B_LOC]  # [T, B_LOC, NKV, D]
        kvT_i = np.ascontiguousarray(
            kv_i.transpose(3, 0, 1, 2).reshape(D, T * B_LOC * NKV)
        )
        in_maps.append({"qT": q_c, "kvT": kvT_i, "wgT": wgT})
    return in_maps


def kernel(q: np.ndarray, kv: np.ndarray, Wg: np.ndarray) -> np.ndarray:
    global _CACHED_NC
    if _CACHED_NC is None:
        _CACHED_NC = build_kernel()
    nc = _CACHED_NC

    in_maps = _make_in_maps(q, kv, Wg)
    res = run_bass_kernel_spmd(nc, in_maps, core_ids=list(range(N_CORES)))
    m_u8 = np.concatenate([r["out"] for r in res.results], axis=1)  # [T,B,D,NQ]
    spikes = (m_u8 == 0)  # device emits NOT-spike
    return np.ascontiguousarray(spikes.transpose(0, 1, 3, 2)).astype(np.float32)


if __name__ == "__main__":
    rng = np.random.default_rng(0)
    q = rng.standard_normal((T, B, NQ, D), dtype=np.float32)
    kv = rng.standard_normal((T, B, NKV, D), dtype=np.float32)
    Wg = (rng.standard_normal((D, D), dtype=np.float32) / np.sqrt(D)).astype(np.float32)
    o = kernel(q, kv, Wg)
    print("out", o.shape, o.dtype, "mean", o.mean())


# revision 20
# speedup vs baseline: 1.6679x; 1.0630x over previous
"""Trainium2 Bass kernel for nn_MemoryUpdate (gated LIF memory update), v3.

Reference computation (fp32):
    k         = einsum('tbnd,od->tbno', kv, Wg)          # kv @ Wg^T
    gate_mean = mean_t'( k[t', b, nkv, d] )              # [Nkv, B, 1, D], Nkv == T
    update    = gate_mean[t, b, d] * q[t, b, n, d]       # broadcast over n
    spikes    = LIF over t: v' = (v + u)/2 ; s = v' >= 0.5 ; v = v' * (1 - s)

Shapes: q [4, 32, 1024, 512], kv [4, 32, 4, 512], Wg [512, 512] -> out [4, 32, 1024, 512].

v3 strategy (v1 was ~238 us, near the fp32 HBM roofline of ~187 us/core;
v3 measures ~96 us): cut HBM traffic 2.7x, use 16-bit DVE modes, keep every
engine's critical chain short.
  - d on partitions: host transposes q to [T, B, D, NQ]. The gate becomes a
    per-partition scalar, so the gate multiply fuses into the recurrence as
    scalar_tensor_tensor((q * g[P,1]) + w) — no broadcast tiles, no gate DMA.
  - q quantized host-side to int16 (scale 6000, max|q*S1| < 32767): halves
    the dominant read. Output written as 1 byte/elem: per-core traffic
    33.5+33.5 MB -> 16.8+8.4 MB (dma-only ablation: 75.6 us).
  - int16 state with a doubling scale schedule S2[t] = 8192 << t: w_i stored
    at scale S2 *is* v/2 at scale 2*S2 = S2[t+1], so the LIF halving is free,
    each step has exactly one int16 rounding site (round-half-even, saturating
    — HW-verified), and precision doubles every step. The merged stt must sum
    g*q + w BEFORE the saturating cast (split u/a saturates u alone: 191K
    flips). Positive clips can't change the spike bit; negative clips are
    rare tails. t=3 uses virtual scale 65536 via threshold 32767.
  - ACT emits m = NOT-spike = (a_i < TH) as int16 {0,1} via saturated
    sigmoid(-1e30*a_i + (TH-0.5)*1e30) — exact (integer gap + true fma,
    HW-verified). int16 m keeps the reset multiply w = a*m on the DVE
    tensor_tensor 2x_1P path (stt is 1x-only: measured 4523 ns vs TT 2327).
    The SWDGE store casts m to uint8 bytes; the HOST inverts (s = m == 0).
  - per-op rates (HW-measured, [128,4096] int16): ts 4x 1.2us, TT 2x 2.3us,
    stt 1x 4.5us, ACT 3.7us. Per core: DVE ~92 us (48 stt-chunk merged ops
    58 + 12 TT resets 26 + 16 t0 ts 6), ACT ~59 us, DMA ~76 us. The reset TT
    is software-pipelined one tile behind its ACT m-pass (in-order engines).
Flip count vs fp32 reference: 463 of 67.1M outputs (rel 1.46e-2 < 2e-2 gate),
bit-identical to the numpy emulation of the int16 pipeline.
"""

import sys

for p in ("/opt/trn_rl_repo", "/root/.axon_site/_ro/trn_rl_repo"):
    if p not in sys.path:
        sys.path.insert(0, p)

import numpy as np

import concourse.bass as bass
import concourse.mybir as mybir
import concourse.tile as tile
from concourse import bacc
from concourse.bass_utils import run_bass_kernel_spmd

# Problem constants (hardcoded per harness contract)
T, B, NQ, NKV, D = 4, 32, 1024, 4, 512
N_CORES = 8
B_LOC = B // N_CORES  # 4
P = 128               # partitions
C = D // P            # 4 d-chunks
NI = T * B_LOC * NKV  # 64 kvT columns
NG = B_LOC * NKV      # 16 gate rows per core

S1 = 6000.0                                    # q fixed-point scale
S2 = [8192.0, 16384.0, 32768.0, 65536.0]       # state scale per step (doubling)
TH = [4096.0, 8192.0, 16384.0, 32767.0]        # spike threshold in a_i units
BIG = 1.0e30

FP32 = mybir.dt.float32
I16 = mybir.dt.int16
U8 = mybir.dt.uint8
Alu = mybir.AluOpType
UNROLL = 8  # static inner unroll inside the timing-mode For_i loop


def build_kernel(repeats=1, timing_mode=False, ablate=frozenset()):
    nc = bacc.Bacc("TRN2", target_bir_lowering=False, debug=False,
                   num_devices=N_CORES)

    if timing_mode:
        # timing-only variant: big tensors live in internal DRAM so the wall
        # clock isn't dominated by host<->device transfers; the main body runs
        # `repeats` times in an on-device loop.
        qT = nc.dram_tensor("q_int", [T, B_LOC, D, NQ], I16).ap()
        out = nc.dram_tensor("out_int", [T, B_LOC, D, NQ], U8).ap()
        dummy = nc.dram_tensor("tiny_out", [P, 16], FP32, kind="ExternalOutput").ap()
    else:
        qT = nc.dram_tensor("qT", [T, B_LOC, D, NQ], I16, kind="ExternalInput").ap()
        out = nc.dram_tensor("out", [T, B_LOC, D, NQ], U8, kind="ExternalOutput").ap()
        dummy = None
    kvT = nc.dram_tensor("kvT", [D, NI], FP32, kind="ExternalInput").ap()
    # wgT is Wg^T host-scaled by 1/(2*T*S1): folds the t'-mean, the LIF /tau
    # and the int16 q descale into the gate.
    wgT = nc.dram_tensor("wgT", [D, D], FP32, kind="ExternalInput").ap()

    # d = c*128 + p on partitions; free = (c, n) -> per-partition runs of
    # 2 KiB (int16 q) / 1 KiB (uint8 out), 1 MiB / 0.5 MiB per DMA.
    qT_v = qT.rearrange("t b (c p) n -> t b p c n", p=P)
    out_v = out.rearrange("t b (c p) n -> t b p c n", p=P)
    kvT_v = kvT.rearrange("(c p) i -> p c i", p=P)
    wgT_v = wgT.rearrange("(c p) o -> p c o", p=P)

    with tile.TileContext(nc) as tc:
        with (
            tc.tile_pool(name="const", bufs=1) as const_pool,
            tc.tile_pool(name="qp", bufs=8) as q_pool,
            tc.tile_pool(name="wp", bufs=8) as w_pool,
            tc.tile_pool(name="mp", bufs=6) as m_pool,
            tc.tile_pool(name="psg", bufs=1, space="PSUM") as psg_pool,
        ):
            # ---- gate computation (all tiny) ----
            kvT_sb = const_pool.tile([P, C * NI], FP32, tag="kvT")
            nc.sync.dma_start(kvT_sb[:].rearrange("p (c i) -> p c i", c=C), kvT_v)
            wgT_sb = const_pool.tile([P, C * D], FP32, tag="wgT")
            nc.sync.dma_start(wgT_sb[:].rearrange("p (c o) -> p c o", c=C), wgT_v)

            # sum over t' of kvT (free layout per chunk: i = t'*16 + (b*4+nkv))
            kv4 = kvT_sb[:].rearrange("p (c tp i) -> p c tp i", c=C, tp=T)
            t01 = const_pool.tile([P, C * NG], FP32, tag="t01")
            t23 = const_pool.tile([P, C * NG], FP32, tag="t23")
            kvs = const_pool.tile([P, C * NG], FP32, tag="kvs")
            t01v = t01[:].rearrange("p (c i) -> p c i", c=C)
            t23v = t23[:].rearrange("p (c i) -> p c i", c=C)
            kvs_v = kvs[:].rearrange("p (c i) -> p c i", c=C)
            nc.vector.tensor_tensor(t01v, kv4[:, :, 0, :], kv4[:, :, 1, :], Alu.add)
            nc.vector.tensor_tensor(t23v, kv4[:, :, 2, :], kv4[:, :, 3, :], Alu.add)
            nc.vector.tensor_tensor(kvs_v, t01v, t23v, Alu.add)

            # g_raw[o_p, (co, b, nkv)] = sum_d wgT[d, o] * kvs[d, (b, nkv)]
            wgv = wgT_sb[:].rearrange("p (c o) -> p c o", c=C)
            psum_g = psg_pool.tile([P, C * NG], FP32)
            pg = psum_g[:].rearrange("p (c i) -> p c i", c=C)
            for co in range(C):
                for dc in range(C):
                    nc.tensor.matmul(
                        pg[:, co, :],
                        wgv[:, dc, co * P:(co + 1) * P],
                        kvs_v[:, dc, :],
                        start=(dc == 0), stop=(dc == C - 1),
                    )
            g_raw = const_pool.tile([P, C * NG], FP32, tag="graw")
            nc.scalar.copy(g_raw[:], psum_g[:])

            # per-step scaled gates: gs[p, (t, c, b)] = g_raw[p, (c, b, t)] * S2[t]
            graw_v = g_raw[:].rearrange("p (c b n) -> p c b n", c=C, b=B_LOC)
            g_s = const_pool.tile([P, T * C * B_LOC], FP32, tag="gs")
            gs_v = g_s[:].rearrange("p (t c b) -> p t c b", t=T, c=C)
            for t in range(T):
                nc.vector.tensor_scalar(
                    gs_v[:, t, :, :], graw_v[:, :, :, t], float(S2[t]), None,
                    Alu.mult,
                )

            # sigmoid threshold biases, one [P,1] per step
            biases = []
            for t in range(T):
                bt = const_pool.tile([P, 1], FP32, tag=f"bias{t}")
                nc.vector.memset(bt[:], float((TH[t] - 0.5) * BIG))
                biases.append(bt)

            if timing_mode:
                junk = const_pool.tile([P, C * NQ], I16, tag="junk")
                nc.vector.memset(junk[:], 0)
                junk4 = junk[:].rearrange("p (c n) -> p c n", c=C)
                for t in range(T):
                    for b in range(B_LOC):
                        nc.sync.dma_start(qT_v[t, b], junk4)
                nc.sync.dma_start(dummy, wgT_sb[:, :16])  # satisfy external output

            import contextlib
            if timing_mode and repeats > 1:
                assert repeats % UNROLL == 0
                rep_ctx = tc.For_i(0, repeats // UNROLL, 1)
                inner_reps = UNROLL
            else:
                rep_ctx = contextlib.nullcontext()
                inner_reps = 1

            # ---- main loop ----
            with rep_ctx:
             for _inner in range(inner_reps):
              w_prev = [None] * B_LOC
              pending_w = None  # deferred reset TT: emitted after the next
                                # tile's stt group so DVE never waits on ACT
              for t in range(T):
                for b in range(B_LOC):
                    qt = q_pool.tile([P, C * NQ], I16, tag="q")
                    q4 = qt[:].rearrange("p (c n) -> p c n", c=C)
                    if "noload" not in ablate:
                        nc.sync.dma_start(q4, qT_v[t, b])
                    if t == 0:
                        # a = g*q in place (tensor_scalar, 4x int16). Keeping
                        # t0 on DVE beats ACT here: an ACT t0 burst serializes
                        # the in-order per-b chains (measured 98 vs 91 us).
                        at = qt
                        if "nodve" not in ablate:
                            for c in range(C):
                                nc.vector.tensor_scalar(
                                    q4[:, c, :], q4[:, c, :],
                                    gs_v[:, 0, c, b:b + 1], None, Alu.mult,
                                )
                    else:
                        at = qt
                        if "nodve" not in ablate:
                            # a = g*q + w, in place over q (stt, 1x but single
                            # rounding site; the pre-saturation sum is required)
                            w4 = w_prev[b][:].rearrange("p (c n) -> p c n", c=C)
                            for c in range(C):
                                nc.vector.scalar_tensor_tensor(
                                    q4[:, c, :], q4[:, c, :],
                                    gs_v[:, t, c, b:b + 1], w4[:, c, :],
                                    Alu.mult, Alu.add,
                                )
                    if pending_w is not None:
                        pending_w()
                        pending_w = None
                    # m = (a_i < TH) as exact 0/1 int16 via saturated sigmoid
                    # (the NOT-spike: host inverts; int16 keeps the reset TT at 2x)
                    mt = m_pool.tile([P, C * NQ], I16, tag="m")
                    if "noact" not in ablate:
                        nc.scalar.activation(
                            mt[:], at[:], mybir.ActivationFunctionType.Sigmoid,
                            bias=biases[t][:], scale=-BIG,
                        )
                    if "nostore" not in ablate:
                        # SWDGE store casts int16 {0,1} -> uint8 bytes
                        src_t = junk if ("noact" in ablate and timing_mode) else mt
                        nc.gpsimd.dma_start(
                            out_v[t, b], src_t[:].rearrange("p (c n) -> p c n", c=C))
                    if "nodve" not in ablate and t < T - 1:
                        # w = a * m  (hard reset; tensor_tensor, 2x int16)
                        wt = w_pool.tile([P, C * NQ], I16, tag="w")

                        def emit_w(wt=wt, at=at, mt=mt):
                            nc.vector.tensor_tensor(wt[:], at[:], mt[:], Alu.mult)

                        pending_w = emit_w
                        w_prev[b] = wt
              if pending_w is not None:
                  pending_w()
                  pending_w = None
    nc.compile()
    return nc


_CACHED_NC = None


def _make_in_maps(q, kv, Wg):
    q = np.ascontiguousarray(q, dtype=np.float32)
    kv = np.ascontiguousarray(kv, dtype=np.float32)
    Wg = np.ascontiguousarray(Wg, dtype=np.float32)

    # transpose so d lands on partitions; quantize q to int16 fixed point
    qT = np.transpose(q, (0, 1, 3, 2))  # [T, B, D, NQ]
    q_i = np.clip(np.rint(qT * np.float32(S1)), -32767, 32767).astype(np.int16)
    wgT = (np.ascontiguousarray(Wg.T) * np.float32(1.0 / (2.0 * T * S1)))
    wgT = np.ascontiguousarray(wgT, dtype=np.float32)

    in_maps = []
    for i in range(N_CORES):
        b0 = i * B_LOC
        q_c = np.ascontiguousarray(q_i[:, b0:b0 + B_LOC])
        kv_i = kv[:, b0:b0 + B_LOC]  # [T, B_LOC, NKV, D]
        kvT_i = np.ascontiguousarray(
            kv_i.transpose(3, 0, 1, 2).reshape(D, T * B_LOC * NKV)
        )
        in_maps.append({"qT": q_c, "kvT": kvT_i, "wgT": wgT})
    return in_maps


def kernel(q: np.ndarray, kv: np.ndarray, Wg: np.ndarray) -> np.ndarray:
    global _CACHED_NC
    if _CACHED_NC is None:
        _CACHED_NC = build_kernel()
    nc = _CACHED_NC

    in_maps = _make_in_maps(q, kv, Wg)
    res = run_bass_kernel_spmd(nc, in_maps, core_ids=list(range(N_CORES)))
    m_u8 = np.concatenate([r["out"] for r in res.results], axis=1)  # [T,B,D,NQ]
    spikes = (m_u8 == 0)  # device emits NOT-spike
    return np.ascontiguousarray(spikes.transpose(0, 1, 3, 2)).astype(np.float32)


if __name__ == "__main__":
    rng = np.random.default_rng(0)
    q = rng.standard_normal((T, B, NQ, D), dtype=np.float32)
    kv = rng.standard_normal((T, B, NKV, D), dtype=np.float32)
    Wg = (rng.standard_normal((D, D), dtype=np.float32) / np.sqrt(D)).astype(np.float32)
    o = kernel(q, kv, Wg)
    print("out", o.shape, o.dtype, "mean", o.mean())
